# revision 53
# baseline (speedup 1.0000x reference)
"""Distributed Trainium2 Bass kernel for multi-head attention.

Reference computation (B=4, S=2048, D=1024, H=16 heads, HD=64):
    q = heads(Q @ Wq + bq + Q_lev)
    k = heads(K @ Wk + bk + K_lev)
    v = heads(V @ Wv + bv + V_lev)
    out = softmax(q k^T / sqrt(HD)) v  -> merge heads -> @ Wo + bo

Sharding: 8 cores = 4 batches x 2 head-halves (tensor parallel on the 16
heads: Wq/Wk/Wv split column-wise, Wo row-wise). Each core computes all
2048 queries for its 8 heads and a PARTIAL output [2048, 1024] = ctx_half
@ Wo_half (bf16); the host sums the two partials of each batch (+bo)
during the unshard. No duplicated projection compute (the query-split
alternative recomputes the K/V projections on both cores of a pair,
+17% PE work) and no on-device collectives.

Device-side layout (feature-major / pre-transposed on the host):
  qT   [HH=512, S]  = Wq_half.T @ Q.T  (+ qlev = (bq + Q_lev).T half)
  kT   [HH, S]      = Wk_half.T @ K.T  (+ klev)
  vaug [tok, 8 heads, 128] = [ones, 63 dead, V @ Wv_half + vlev] so the
        ctx matmul emits the softmax denominator on PSUM row 0 and ctx on
        rows 64..127 (AP partition bases are limited to 0/32/64/96, and a
        base-32 access may span at most 32 partitions)
  scoresT[keys, q] = kT_h.T @ qT_h     (contract over HD=64)
  probsT = exp(scoresT / 8)            (no max subtraction: scores are
                                        N(0,~2) so exp stays < ~1e6)
  ctx_aug[128, q] = vaug_h.T @ probsT
  ctxT = ctx_aug[64:128] * (1/denominator)  (DVE fast reciprocal read
        straight off PSUM row 0, bf16 cast, then a K=1 matmul against a
        ones row broadcasts it across the 64 head-dim partitions)
  out_partial[q, D] = ctxT.T @ Wo_half

Matmuls run in bf16 (f32 PSUM accumulation). The two K=64 scores matmuls
of a head pair run concurrently in PE row halves (tile_position derived
from base partitions 0/64) and write the two banks of one [128, 1024]
PSUM tile so a single wide ACT exp serves both heads.

Engine balance per core (MEASURED on HW, not the original estimates):
the kernel is PE-BOUND: TensorE busy ~350us of ~383us total. 1568
matmuls, all N=512 moving (~216ns warm each); a scores head-pair with
tile_position row halves occupies ~300ns (216 + ~84ns second-tile
LDWEIGHTS exposure -- true co-execution only partial). ScalarE exp is
256 ACTIVATEs x ~1.11us = ~289us ((N+352)/1.2ns; PSUM-src). DVE ~190us.
Schedule:
 - fillers(kc)/scores(kc)/exp(kc)/ctx(kc-LAG) software pipeline per
   key chunk, LAG=3 (LAG=2 exposed ctx fill stalls behind just-finished
   exps); all projections and the output projection are "fillers".
   Fillers run BEFORE scores in each step: scores-e0's LDWEIGHTS waits
   on the sps rotation (exp(kc-2) drain) and the in-order PE queue
   head-of-line blocks anything emitted behind it, so ready fillers
   must precede the wait (-3us). The per-call filler budget is spread
   EVENLY across all 16 kc steps (not ceil-front-loaded, which drains
   the list by ~kc 12 and leaves the last steps' sps-waits uncovered:
   -3us more), and the SUPPLY is balanced at 17 fillers per call for
   qb>=1 (qT m3 moved from the hp0 call to hp1, outproj groups resliced
   2/4/2: -1.6us; the old 25/13/13/17 split left the 13-filler calls
   with uncovered steps). Moving next_first after ctx(13) by the same logic
   measured ~1us WORSE (delays exp(0') too much) -- keep next_first
   right after scores(15).
 - call 1 carries the whole v projection + kT[0] n1-3 + kT[1] + the
   remaining qT(qb0) groups, ordered by DATA ARRIVAL: engine queues are
   in-order at runtime and the static tile scheduler does not model DMA
   latency, so a filler emitted before its input lands head-of-line
   blocks every later scores -> the filler order must match the DMA
   stream order.
 - the input stream (~16MB critical) is issued up front on the sync
   queue in exact first-use order, with the FIRST transfers sub-chunked
   (kin/qin DC-halves interleaved with weight pieces) so the first
   scores/exp fire at ~18us instead of ~30us. lev sideloads ride
   gpsimd. Constant-region memsets (vaug ones/zeros) and a 24-matmul
   HAM warmup run in the DMA-dead launch window.
 - each call's normalize is deferred into the next call's filler
   stream at index 2, and each call's tail borrows the NEXT call's
   first two fillers (run between ctx(14) and ctx(15)) so the PE never
   idles on the exp(15)->ctx(15)->reciprocal chain at call boundaries.
   Normalize = two K=1 M=64 col-tile matmuls filling one PSUM tile
   (partitions 0:64 / 64:128) + a single 128-row DVE multiply.
 - the tail (last query block's outproj) pre-accumulates dc 0..2 for 7
   groups into the idle attention PSUM slots before the final
   normalize; epilogue casts alternate DVE/ScalarE (explicit dep on the
   final exp against HOL), and the 8 output stores rotate across the
   sync/gpsimd/scalar queues (a DMA issue costs ~1.1us of queue time).
 - optional (KF_DVE_EXP=n, default 0): offload exp for n of 16 kc to
   DVE as a Schraudolph bit-trick (int16 bits of bf16 2^y via one
   tensor_scalar mult+add, bitcast to bf16). Works (adds ~1.8%rms
   sawtooth on the offloaded probs, total rel_err 1.08e-2 at n=4) but
   buys ~0 time since the kernel is PE-bound, so it is off.
Measured dead ends (HW): fp8 (accuracy), f32r probs (walrus forbids
mixed 16/32-bit matmul inputs), gpsimd partition_broadcast for the
normalize (sim-ok but races on HW -- not Tile-dep-tracked), standalone
nc.tensor.ldweights prefetch for the scores pair (NaN), ragged-K
matmuls like K=33 (PE tile rounds to 64 and contracts stale rows ->
NaN), splitting the input stream across sync+scalar or sync+gpsimd
queues (no aggregate bandwidth gain, ordering regressions), DMA-split
halves generally (slower ramp), interleaving a filler matmul between
the two ctx matmuls of a pair to hide the second LDWEIGHTS (+13us --
disrupts the accumulation pipeline), extra HAM warm-bursts inside the
ramp's data-wait gaps (net slower). Postamble: ~8-10us of per-engine
semaphore drain after the last real instruction is framework-fixed.
"""

import os
import sys

import numpy as np

for _p in ("/opt/trn_rl_repo", "/root/.axon_site/_ro/trn_rl_repo"):
    if os.path.isdir(_p) and _p not in sys.path:
        sys.path.insert(0, _p)

import ml_dtypes  # noqa: E402

B, S, D, H = 4, 2048, 1024, 16
HD = D // H  # 64
HH = D // 2  # 512 output-feature half per core
NH = H // 2  # 8 heads per core
N_CORES = 8
P = 128  # SBUF partitions
DC = D // P  # 8 chunks of the full (contraction) feature dim
MC = HH // P  # 4 chunks of my output-feature half
KC = S // P  # 16 key chunks
NB = 512  # matmul moving free-dim (one PSUM bank of f32)
NQB = S // NB  # 4 query blocks
CO = 64  # ctx offset inside vaug: [ones, 63 pad, 64 head dims] so the
CW = CO + HD  # denominator lands on PSUM row 0 and ctx on rows 64..127
#              (base-64 spans of 64 partitions are legal APs; a base-32
#              span may only cover 32 partitions. Rows 1..63 are dead.)

_BUILD_CACHE = {}

# bisect flags (default ON; flip via env for debugging only)
F_SCALAR_DMA = os.environ.get("KF_SCALAR_DMA", "1") == "1"
F_SPLIT = os.environ.get("KF_SPLIT", "0") == "1"
F_NORM_MERGE = os.environ.get("KF_NORM_MERGE", "1") == "1"
F_BORROW = os.environ.get("KF_BORROW", "1") == "1"
F_HOIST = os.environ.get("KF_HOIST", "1") == "1"
F_TAILQ = os.environ.get("KF_TAILQ", "1") == "1"
# number of key-chunks per call whose exp runs on the Vector engine as a
# Schraudolph bit-trick (int32 bits of 2^y built with one mult+add, bitcast
# to f32r for the ctx matmul -- f32r moving operands stream at full rate for
# N>=256). Offloading frees ~1.1us of ScalarE per chunk so the exp stream
# stops pacing the pipeline. Constants give exp(s/8) with zero-mean ~1.8%
# rms sawtooth error on the offloaded fraction.
N_DVE_EXP = int(os.environ.get("KF_DVE_EXP", "0"))
F_NORM_DMA = os.environ.get("KF_NORM_DMA", "0") == "1"
F_LDWPRE = os.environ.get("KF_LDWPRE", "0") == "1"
# bf16-bit-space constants: bits16 = round(2^7 * (s*log2e/8 + 127 - 0.05487))
SCHRAUD_A = 128.0 * 1.4426950408889634 / 8.0
SCHRAUD_B = 128.0 * (127.0 - 0.05487)


def _build_nc():
    from concourse import bacc, mybir, tile
    from concourse.bass import _add_dep_helper

    f32 = mybir.dt.float32
    i16 = mybir.dt.int16
    bf16 = mybir.dt.bfloat16
    Exp = mybir.ActivationFunctionType.Exp
    DVE_KC = set([3, 7, 11, 14][:N_DVE_EXP])

    nc = bacc.Bacc("TRN2", target_bir_lowering=False, debug=False, num_devices=N_CORES)

    qt_d = nc.dram_tensor("qt", [D, S], bf16, kind="ExternalInput")
    qlev_d = nc.dram_tensor("qlev", [HH, S], bf16, kind="ExternalInput")
    kt_d = nc.dram_tensor("kt", [D, S], bf16, kind="ExternalInput")
    klev_d = nc.dram_tensor("klev", [HH, S], bf16, kind="ExternalInput")
    vt_d = nc.dram_tensor("vt", [D, S], bf16, kind="ExternalInput")
    vlev_d = nc.dram_tensor("vlev", [S, HH], bf16, kind="ExternalInput")
    wq_d = nc.dram_tensor("wq", [D, HH], bf16, kind="ExternalInput")
    wk_d = nc.dram_tensor("wk", [D, HH], bf16, kind="ExternalInput")
    wv_d = nc.dram_tensor("wv", [D, HH], bf16, kind="ExternalInput")
    wo_d = nc.dram_tensor("wo", [HH, D], bf16, kind="ExternalInput")
    out_d = nc.dram_tensor("out", [S, D], bf16, kind="ExternalOutput")

    # [D, x] dram views as [P, DC, x] (partition-major for merged DMAs)
    qt_v = qt_d.rearrange("(i p) s -> p i s", p=P)
    kt_v = kt_d.rearrange("(i p) s -> p i s", p=P)
    vt_v = vt_d.rearrange("(i p) s -> p i s", p=P)
    wq_v = wq_d.rearrange("(i p) c -> p i c", p=P)
    wk_v = wk_d.rearrange("(i p) c -> p i c", p=P)
    wv_v = wv_d.rearrange("(i p) c -> p i c", p=P)
    wo_v = wo_d.rearrange("(i p) c -> p i c", p=P)

    with tile.TileContext(nc) as tc:
        with (
            tc.tile_pool(name="persist", bufs=1) as persist,
            tc.tile_pool(name="qinp", bufs=2) as qinp,
            tc.tile_pool(name="vinp", bufs=2) as vinp,
            tc.tile_pool(name="lev", bufs=2) as levp,
            tc.tile_pool(name="probs", bufs=4) as prp,
            tc.tile_pool(name="probsf", bufs=2) as prfp,
            tc.tile_pool(name="norm", bufs=1) as nrm,
            tc.tile_pool(name="psum", bufs=1, space="PSUM") as psum,
        ):
            # Persistent intermediates (bf16).
            qT = [persist.tile([P, S], bf16, name=f"qT{i}", tag=f"qT{i}") for i in range(MC)]
            kT = [persist.tile([P, S], bf16, name=f"kT{i}", tag=f"kT{i}") for i in range(MC)]
            vaug = [
                persist.tile([P, NH, CW], bf16, name=f"vaug{i}", tag=f"vaug{i}")
                for i in range(KC)
            ]
            ctxT = [persist.tile([P, S], bf16, name=f"ctxT{i}", tag=f"ctxT{i}") for i in range(MC)]
            # head-indicator rows: partition 0 selects head-A columns
            # (0..63), partition 32 head-B columns (64..127). One K=64
            # matmul against the reciprocal tile (recA on partition 0,
            # recB on 32 — AP partition bases are limited to 0/32/64/96 —
            # all other rows zeroed once; K is a full 64 so the PE tile is
            # exact: a ragged K like 33 rounds the tile to 64 rows and the
            # HW contracts stale/uninitialized data in rows 33..63)
            # broadcasts each head's reciprocal across its 64 head-dim
            # partitions in a single instruction.
            ind_t = persist.tile([1, P], bf16, name="ind_t", tag="ind_t")
            # Merged weight/input tiles: one DMA each (DMA issue is ~600ns
            # per instruction on the issuing queue; the startup is gated on
            # instruction count as much as bytes).
            wk_sb = persist.tile([P, DC, HH], bf16, name="wk", tag="wk")
            wq_sb = persist.tile([P, DC, HH], bf16, name="wq", tag="wq")
            wv_sb = persist.tile([P, DC, HH], bf16, name="wv", tag="wv")
            wo_sb = persist.tile([P, MC, D], bf16, name="wo", tag="wo")
            kin = persist.tile([P, DC, S], bf16, name="kin", tag="kin")

            # ---- DMA ordering ----
            # The projection phase is DMA-bound (~16MB of critical input
            # stream), so the input stream is issued up front in first-use
            # order, SPLIT across two parallel queues (sync + a second
            # engine queue) to push the aggregate rate past a single
            # queue's ~350GB/s toward the ~430GB/s fabric ceiling. Each
            # [P, DC, x] transfer is halved on DC (sync gets chunks 0:4, a
            # helper queue 4:8) so both halves land concurrently and every
            # dep arrives earlier than the single-queue stream would.
            # Queue roles: sync + scalar carry the prelude (scalar's queue
            # is idle until the first exp at ~10us; 4 issues fit before
            # it), gpsimd carries the rest (+ lev sideloads as before);
            # vector issues NO DMAs so projection epilogues never queue
            # behind a ~1us DMA-issue instruction.
            DH = DC // 2
            gp_q = nc.gpsimd if F_SPLIT else nc.sync
            # Two-queue ramp: the early DMA stream is in-flight-limited on
            # a single queue (~200GB/s for the first ~10us), so the K/V
            # stream rides sync while the Q stream + small items ride
            # gpsimd, each in strict first-need order. The two queues'
            # transfers pipeline across the 16 DMA engines concurrently,
            # roughly doubling early arrival rate; first scores/exp fire
            # ~10us earlier than the single-queue ramp.
            klev00 = levp.tile([P, NB], bf16, name="klev00", tag="lev0", bufs=1)
            qlev00 = levp.tile([P, NB], bf16, name="qlev00", tag="lev0b", bufs=1)
            qin = {}

            def load_qin(n, split=False):
                t = qinp.tile([P, DC, NB], bf16, name="qin", tag="qin")
                nc.sync.dma_start(t[:], qt_v[:, :, n * NB : (n + 1) * NB])
                qin[n] = t

            if F_SPLIT:
                # sync: K/V stream
                nc.sync.dma_start(kin[:, 0:DH, 0:NB], kt_v[:, 0:DH, 0:NB])
                nc.sync.dma_start(kin[:, DH:DC, 0:NB], kt_v[:, DH:DC, 0:NB])
                # gpsimd: weights + Q stream
                nc.gpsimd.dma_start(wk_sb[:, :, 0:P], wk_v[:, :, 0:P])
                nc.gpsimd.dma_start(klev00[:], klev_d[0:P, 0:NB])
                nc.gpsimd.dma_start(wq_sb[:, :, 0:P], wq_v[:, :, 0:P])
                t = qinp.tile([P, DC, NB], bf16, name="qin", tag="qin")
                nc.gpsimd.dma_start(t[:, 0:DH, :], qt_v[:, 0:DH, 0:NB])
                nc.gpsimd.dma_start(t[:, DH:DC, :], qt_v[:, DH:DC, 0:NB])
                nc.gpsimd.dma_start(qlev00[:], qlev_d[0:P, 0:NB])
                qin[0] = t
                nc.sync.dma_start(wv_sb[:, 0:DH, :], wv_v[:, 0:DH, :])
                nc.sync.dma_start(wv_sb[:, DH:DC, :], wv_v[:, DH:DC, :])
                nc.gpsimd.dma_start(wq_sb[:, :, P:HH], wq_v[:, :, P:HH])
                nc.sync.dma_start(kin[:, :, NB : 2 * NB], kt_v[:, :, NB : 2 * NB])
            else:
                nc.sync.dma_start(kin[:, 0:DH, 0:NB], kt_v[:, 0:DH, 0:NB])
                nc.sync.dma_start(wk_sb[:, 0:DH, 0:P], wk_v[:, 0:DH, 0:P])
                nc.sync.dma_start(kin[:, DH:DC, 0:NB], kt_v[:, DH:DC, 0:NB])
                nc.sync.dma_start(wk_sb[:, DH:DC, 0:P], wk_v[:, DH:DC, 0:P])
                nc.sync.dma_start(klev00[:], klev_d[0:P, 0:NB])
                t = qinp.tile([P, DC, NB], bf16, name="qin", tag="qin")
                nc.sync.dma_start(t[:, 0:DH, :], qt_v[:, 0:DH, 0:NB])
                nc.sync.dma_start(wq_sb[:, :, 0:P], wq_v[:, :, 0:P])
                nc.sync.dma_start(t[:, DH:DC, :], qt_v[:, DH:DC, 0:NB])
                nc.sync.dma_start(qlev00[:], qlev_d[0:P, 0:NB])
                qin[0] = t
                nc.sync.dma_start(wq_sb[:, :, P:HH], wq_v[:, :, P:HH])
                nc.sync.dma_start(wv_sb[:], wv_v[:])
                nc.sync.dma_start(kin[:, :, NB : 2 * NB], kt_v[:, :, NB : 2 * NB])
            # constant regions of the persistent tiles: fill during the
            # DMA-dead launch window instead of on call 1's critical path
            nc.vector.memset(ind_t[:], 1.0)
            # HAM warmup: the PE clock sits gated at 1.2GHz until ~3.4us of
            # sustained activity. The first ~25us of real matmuls are
            # DMA-paced and run cold (634ns vs 379ns warm). Dummy K=1
            # matmuls on the ones row (no data deps) warm the clock during
            # the DMA-dead launch window so the real ramp runs at 2.4GHz.
            warm_ps = psum.tile([P, NB], f32, name="warm", tag="ps_proj", bufs=2)
            for wi in range(24):
                nc.tensor.matmul(
                    warm_ps[0:1, 0:P], ind_t[0:1, 0:1], ind_t[0:1, :],
                    start=(wi == 0), stop=(wi == 23),
                )
            if F_HOIST:
                for m in range(KC):
                    nc.vector.memset(vaug[m][:, :, 0:CO], 0.0)
                    nc.vector.memset(vaug[m][:, :, 0:1], 1.0)
            vin = {}

            # ---------------- projection fillers -------------
            def kT_chunk_fillers(m, n0=0, lev_pre=None):
                """kT[m] = Wk[:, m-chunk].T @ K.T: psum groups of 8
                accumulating matmuls + DVE epilogue each."""
                state = {}
                fillers = []
                for n in range(n0, NQB):
                    for kc in range(DC):
                        def mmf(n=n, kc=kc):
                            if kc == 0:
                                state[n] = psum.tile(
                                    [P, NB], f32, name="psk", tag="ps_proj", bufs=2
                                )
                                if lev_pre and n in lev_pre:
                                    state["lev", n] = lev_pre[n]
                                else:
                                    lev = levp.tile([P, NB], bf16, name="levk", tag="lev")
                                    nc.gpsimd.dma_start(
                                        lev[:],
                                        klev_d[m * P : (m + 1) * P, n * NB : (n + 1) * NB],
                                    )
                                    state["lev", n] = lev
                            nc.tensor.matmul(
                                state[n][:],
                                wk_sb[:, kc, m * P : (m + 1) * P],
                                kin[:, kc, n * NB : (n + 1) * NB],
                                start=(kc == 0),
                                stop=(kc == DC - 1),
                            )
                            if kc == DC - 1:
                                nc.vector.tensor_add(
                                    kT[m][:, n * NB : (n + 1) * NB],
                                    state[n][:],
                                    state["lev", n][:],
                                )
                        fillers.append(mmf)
                return fillers

            def qT_group_fillers(m, n, lev_t=None):
                state = {}
                fillers = []
                for kc in range(DC):
                    def mmf(kc=kc):
                        if kc == 0:
                            state[0] = psum.tile(
                                [P, NB], f32, name="psq", tag="ps_proj", bufs=2
                            )
                        nc.tensor.matmul(
                            state[0][:],
                            wq_sb[:, kc, m * P : (m + 1) * P],
                            qin[n][:, kc, :],
                            start=(kc == 0),
                            stop=(kc == DC - 1),
                        )
                        if kc == DC - 1:
                            if lev_t is not None:
                                lev = lev_t
                            else:
                                lev = levp.tile([P, NB], bf16, name="levq", tag="lev")
                                nc.gpsimd.dma_start(
                                    lev[:],
                                    qlev_d[m * P : (m + 1) * P, n * NB : (n + 1) * NB],
                                )
                            nc.vector.tensor_add(
                                qT[m][:, n * NB : (n + 1) * NB], state[0][:], lev[:]
                            )
                    fillers.append(mmf)
                return fillers

            # v projection: vaug[m] (tokens m*128..) = V @ Wv_half + vlev,
            # head-strided with ones columns. 8 matmuls per chunk.
            vin = {}
            vstate = {}
            vlev_t = {}

            def load_vlev(m):
                t = levp.tile([P, NB], bf16, name="vlev", tag="vlev", bufs=4)
                nc.gpsimd.dma_start(t[:], vlev_d[m * P : (m + 1) * P, :])
                vlev_t[m] = t

            def v_chunk_fillers(m):
                c = m // 4
                fillers = []
                for kc in range(DC):
                    def mmf(kc=kc, m=m, c=c):
                        if kc == 0 and m % 4 == 0:
                            t = vinp.tile([P, DC, NB], bf16, name="vin", tag="vin")
                            if F_SPLIT:
                                dh = DC // 2
                                nc.sync.dma_start(
                                    t[:, 0:dh, :], vt_v[:, 0:dh, c * NB : (c + 1) * NB]
                                )
                                gp_q.dma_start(
                                    t[:, dh:DC, :], vt_v[:, dh:DC, c * NB : (c + 1) * NB]
                                )
                            else:
                                nc.sync.dma_start(t[:], vt_v[:, :, c * NB : (c + 1) * NB])
                            vin[c] = t
                        if kc == 0:
                            # vlev prefetched ~3 chunks ahead so the
                            # epilogue add never waits on the transfer
                            if m == 0:
                                for mm_ in range(min(4, KC)):
                                    load_vlev(mm_)
                            elif m + 3 < KC:
                                load_vlev(m + 3)
                        if kc == 0:
                            vstate[0] = psum.tile(
                                [P, NB], f32, name="psv", tag="ps_proj", bufs=2
                            )
                        nc.tensor.matmul(
                            vstate[0][:],
                            vin[c][:, kc, (m % 4) * P : (m % 4 + 1) * P],
                            wv_sb[:, kc, :],
                            start=(kc == 0),
                            stop=(kc == DC - 1),
                        )
                        if kc == DC - 1:
                            nc.vector.tensor_add(
                                vaug[m][:, :, CO:CW],
                                vstate[0][:].rearrange("p (h d) -> p h d", h=NH),
                                vlev_t[m][:].rearrange("p (h d) -> p h d", h=NH),
                            )
                            if not F_HOIST:
                                nc.vector.memset(vaug[m][:, :, 0:CO], 0.0)
                                nc.vector.memset(vaug[m][:, :, 0:1], 1.0)
                    fillers.append(mmf)
                return fillers

            def run_fillers(fillers, k):
                for _ in range(min(k, len(fillers))):
                    fillers.pop(0)()

            last_act = {}

            def emit_scores_exp(qb, hp, kc):
                qs = slice(qb * NB, (qb + 1) * NB)
                sps = psum.tile([P, 2 * NB], f32, name="sps", tag="sps", bufs=2)
                if F_LDWPRE:
                    # pre-load the second row-half tile's weights so its
                    # matmul streams concurrently with the first instead of
                    # trailing by the weight-load
                    nc.tensor.ldweights(
                        kT[hp][HD:P, kc * P : (kc + 1) * P], tile_position=(64, 0)
                    )
                for e in range(2):
                    rows = slice(e * HD, (e + 1) * HD)
                    # head pair packed in PE row halves
                    nc.tensor.matmul(
                        sps[:, e * NB : (e + 1) * NB],
                        kT[hp][rows, kc * P : (kc + 1) * P],
                        qT[hp][rows, qs],
                        start=True,
                        stop=True,
                    )
                if kc in DVE_KC:
                    # Schraudolph fake-exp on the Vector engine: int16 bits
                    # of bf16(2^(s*log2e/8)), read back as bf16 by the ctx
                    # matmul (walrus forbids mixed 32/16-bit matmul inputs)
                    prf = prfp.tile([P, 2 * NB], i16, name="prf", tag="prf")
                    nc.vector.tensor_scalar(
                        prf[:], sps[:], SCHRAUD_A, SCHRAUD_B,
                        mybir.AluOpType.mult, mybir.AluOpType.add,
                    )
                    return (prf, True)
                pr = prp.tile([P, 2 * NB], bf16, name="pr", tag="pr")
                ai = nc.scalar.activation(pr[:], sps[:], Exp, scale=1.0 / 8.0)
                last_act["ai"] = ai
                return (pr, False)

            def emit_attention(
                qb, hp, fillers=None, per_kc=3, hooks=None, sched=None,
                pre=None, next_first=None, tail_fillers=(),
            ):
                qs = slice(qb * NB, (qb + 1) * NB)
                fillers = fillers if fillers is not None else []
                hooks = hooks or {}
                cps = [
                    psum.tile([CW, NB], f32, name=f"cps{e}", tag="ctxps", bufs=2)
                    for e in range(2)
                ]
                # software pipeline per kc: scores(kc); exp(kc); PE filler
                # work (projections/outproj) in the exp-wait gap; ctx(kc-2)
                # (lag 2 so ctx never waits on the just-issued exp; probs
                # bufs=4 covers the extra in-flight tile). tail_fillers are
                # the NEXT call's first two fillers (dependency-safe: its
                # own norm filler is inserted after them), run right before
                # the final ctx pair: ctx(15) trails exp(15) by ~1.1us of
                # ScalarE time while the PE has only scores(0')+ctx(14)
                # (~0.75us) to chew on, so without them the PE idles
                # ~0.4us every call.
                LAG = 3
                prs = {}
                nxt_pre = None
                for kc in range(KC + LAG):
                    if kc < KC:
                        # fillers BEFORE scores: scores-e0's LDWEIGHTS
                        # waits on the sps buffer (exp(kc-2) drain) and the
                        # in-order PE queue head-of-line blocks everything
                        # behind it, so ready fillers must be emitted ahead
                        # of the wait, not after it
                        run_fillers(fillers, sched[kc] if sched else per_kc)
                        if kc == 0 and pre is not None:
                            # scores(0)/exp(0) were already emitted inside
                            # the previous call's drain (cross-call exp
                            # pipelining)
                            prs[0] = pre
                        else:
                            prs[kc] = emit_scores_exp(qb, hp, kc)
                        if kc == KC - 1 and next_first is not None:
                            # emit the NEXT call's first scores/exp before
                            # this call's last THREE ctx pairs (all fillers
                            # have drained by now, so the next chunk's
                            # kT/qT are fully emitted), keeping the scalar
                            # queue fed across the call boundary
                            nxt_pre = next_first()
                    if kc >= LAG:
                        pkc = kc - LAG
                        if pkc == KC - 1:
                            run_fillers(fillers, len(fillers))
                            for tf in tail_fillers:
                                tf()
                        ppr, is_dve = prs.pop(pkc)
                        for e in range(2):
                            mv = ppr[:, e * NB : (e + 1) * NB]
                            if is_dve:
                                mv = mv.bitcast(bf16)
                            nc.tensor.matmul(
                                cps[e][:],
                                vaug[pkc][:, 2 * hp + e, :],
                                mv,
                                start=(pkc == 0),
                                stop=(pkc == KC - 1),
                            )
                run_fillers(fillers, len(fillers))
                # 1/denominator straight off PSUM row 0 — no staging copy
                # or DMA. Both heads' reciprocals land on partitions 0/32
                # of the persistent zeroed rec tile so the bcast is a
                # single K=64 matmul.
                recbs = []
                for e in range(2):
                    recf = nrm.tile([1, NB], f32, name=f"recf{e}", tag=f"rf{e}", bufs=1)
                    nc.vector.reciprocal_approx_fast(recf[:], cps[e][0:1, :])
                    recb = nrm.tile([1, NB], bf16, name=f"recb{e}", tag=f"rb{e}", bufs=1)
                    nc.vector.tensor_copy(recb[:], recf[:])
                    recbs.append(recb)
                for e in range(2):
                    rows = slice(e * HD, (e + 1) * HD)
                    # copy unnormalized ctx (normalized in place later)
                    nc.vector.tensor_copy(ctxT[hp][rows, qs], cps[e][CO:CW, :])
                return (qb, hp, recbs), nxt_pre

            def emit_norm_finish(pend):
                # Normalize a head pair (deferred into the next call's
                # filler stream): ONE K=2 matmul against the head-indicator
                # rows broadcasts head A's reciprocal onto partitions 0..63
                # and head B's onto 64..127, then one 128-row multiply.
                qb, hp, recbs = pend
                qs = slice(qb * NB, (qb + 1) * NB)
                if F_NORM_DMA:
                    # broadcast each head's reciprocal across its 64
                    # head-dim partitions with the DMA engine's
                    # partition-broadcast (gpsimd queue has slack) -- no PE
                    # matmul, and the multiply reads SBUF bf16 at 2x
                    bcb = nrm.tile([P, NB], bf16, name="bcb", tag="bcb", bufs=1)
                    nc.gpsimd.partition_broadcast(bcb[0:HD, :], recbs[0][:], channels=HD)
                    nc.gpsimd.partition_broadcast(bcb[HD:P, :], recbs[1][:], channels=HD)
                    nc.vector.tensor_mul(ctxT[hp][:, qs], ctxT[hp][:, qs], bcb[:])
                elif F_NORM_MERGE:
                    # two K=1 M=64 col-tiles (out partitions 0:64 / 64:128)
                    # fill ONE psum tile so a single 128-row DVE multiply
                    # normalizes both heads at once
                    bc = psum.tile([P, NB], f32, name="bc", tag="ps_proj", bufs=2)
                    nc.tensor.matmul(
                        bc[0:HD, :], ind_t[0:1, 0:HD], recbs[0][:], start=True, stop=True
                    )
                    nc.tensor.matmul(
                        bc[HD:P, :], ind_t[0:1, 0:HD], recbs[1][:], start=True, stop=True
                    )
                    nc.vector.tensor_mul(ctxT[hp][:, qs], ctxT[hp][:, qs], bc[:])
                else:
                    for e in range(2):
                        rows = slice(e * HD, (e + 1) * HD)
                        bc = psum.tile([P, NB], f32, name="bc", tag="ps_proj", bufs=2)
                        nc.tensor.matmul(
                            bc[:], ind_t[0:1, :], recbs[e][:], start=True, stop=True
                        )
                        nc.vector.tensor_mul(
                            ctxT[hp][rows, qs], ctxT[hp][rows, qs], bc[0:HD, :]
                        )

            def outproj_fillers(qg, n):
                state = {}
                fillers = []
                for dc in range(MC):
                    def mmf(dc=dc):
                        if dc == 0:
                            state[0] = psum.tile(
                                [P, NB], f32, name="pso", tag="ps_proj", bufs=2
                            )
                        nc.tensor.matmul(
                            state[0][:],
                            ctxT[dc][:, qg * P : (qg + 1) * P],
                            wo_sb[:, dc, n * NB : (n + 1) * NB],
                            start=(dc == 0),
                            stop=(dc == MC - 1),
                        )
                        if dc == MC - 1:
                            ot = nrm.tile([P, NB], bf16, name="ot", tag="otile", bufs=4)
                            nc.vector.tensor_copy(ot[:], state[0][:])
                            nc.sync.dma_start(
                                out_d[qg * P : (qg + 1) * P, n * NB : (n + 1) * NB],
                                ot[:],
                            )
                    fillers.append(mmf)
                return fillers

            # ---- interleaved schedule ----
            # Projection-first: the projection phase is DMA-bound anyway, so
            # kT[0], qT[*][qb0] and the whole v projection run before
            # attention, with the PE trailing the input stream at full DMA
            # bandwidth and ScalarE idle. From call 1 on, every attention
            # call is exp-paced (~17.8us) and carries light PE fillers
            # (next kT chunk / next qT groups / previous query block's
            # output projection) in its exp-wait gaps.
            kt0 = kT_chunk_fillers(0, lev_pre={0: klev00})
            for f in kt0[0:DC]:  # n-block 0 (kin-n0 + wk-c0 only)
                f()
            for f in qT_group_fillers(0, 0, lev_t=qlev00):
                f()

            # Call 1 (qb0, hp0) carries every remaining projection as PE
            # fillers, in data-arrival order (engine queues are in-order at
            # runtime and the static scheduler does not model DMA latency,
            # so emission order must match the transfer stream). ScalarE
            # starts its exp stream ~15us in. Deadlines: kT[0] n-block j
            # fully emitted at least one step before scores(4j); vaug[m]
            # fully emitted by step m+LAG (the ctx emission point); the
            # lazy vt/kin/wk DMA emissions land on the sync queue between
            # the matching fillers.
            vch = [v_chunk_fillers(m) for m in range(KC)]
            c1 = []
            for m in range(1, MC):
                c1 += qT_group_fillers(m, 0)          # pos 0-23
            DHs = DC // 2

            def split_dma(dst_a, src_a, dst_b, src_b):
                nc.sync.dma_start(dst_a, src_a)
                gp_q.dma_start(dst_b, src_b)

            c1 += vch[0]                              # pos 24-31 (vt-c0 dma)
            c1 += kt0[DC : 2 * DC]                    # pos 32-39: kT0-n1
            c1.append(lambda: split_dma(
                kin[:, 0:DHs, 2 * NB : 3 * NB], kt_v[:, 0:DHs, 2 * NB : 3 * NB],
                kin[:, DHs:DC, 2 * NB : 3 * NB], kt_v[:, DHs:DC, 2 * NB : 3 * NB]))
            c1 += vch[1] + vch[2] + vch[3] + vch[4]   # pos 41-72 (vt-c1 @ 65)
            c1 += kt0[2 * DC : 3 * DC]                # pos 73-80: kT0-n2
            c1.append(lambda: split_dma(
                kin[:, 0:DHs, 3 * NB : S], kt_v[:, 0:DHs, 3 * NB : S],
                kin[:, DHs:DC, 3 * NB : S], kt_v[:, DHs:DC, 3 * NB : S]))
            c1 += vch[5] + vch[6] + vch[7]            # pos 82-105
            c1 += kt0[3 * DC :]                       # pos 106-113: kT0-n3
            c1 += vch[8] + vch[9] + vch[10] + vch[11] + vch[12]  # vt-c2/c3
            c1.append(lambda: split_dma(
                wk_sb[:, 0:DHs, P:HH], wk_v[:, 0:DHs, P:HH],
                wk_sb[:, DHs:DC, P:HH], wk_v[:, DHs:DC, P:HH]))
            c1 += vch[13] + vch[14] + vch[15]
            c1 += kT_chunk_fillers(1)                 # call 2's kT chunk

            pend = None
            pre = None
            calls = [(qb, hp) for qb in range(NQB) for hp in range(MC)]

            kt3_tail = {}

            def build_fillers(idx):
                qb, hp = calls[idx]
                fillers = []
                if qb == 0:
                    if hp == 0:
                        fillers += c1
                    if hp == 1:
                        fillers += kT_chunk_fillers(2)
                    if hp == 2:
                        # kT[3]'s last n-block moves to call (0,3), which
                        # otherwise has only 9 fillers (7 kc steps without
                        # sps-wait cover). Its deadline is scores(0,3,12),
                        # and it sits at positions 0-7 there (emitted by
                        # ~kc 7 under the even spread).
                        kt3 = kT_chunk_fillers(3)
                        kt3_tail["n3"] = kt3[3 * DC :]
                        fillers += kt3[: 3 * DC]
                    if hp == 3:
                        fillers += kt3_tail.pop("n3")
                else:
                    # balance the filler supply at 17 per call (16 kc steps
                    # each want >=1 filler of sps-wait cover): hp0 keeps qT
                    # m1-2, hp1 takes qT m3 + 2 outproj groups, hp2 takes 4
                    # groups, hp3 takes 2 (+ the prepended next-qb qT(0))
                    if hp == 0:
                        for m in (1, 2):
                            fillers += qT_group_fillers(m, qb)
                    else:
                        if hp == 1:
                            fillers += qT_group_fillers(3, qb)
                        # previous qb's outproj: 8 groups over 3 calls
                        og = [(4 * (qb - 1) + g, n) for g in range(4) for n in range(2)]
                        take = {1: og[0:2], 2: og[2:6], 3: og[6:8]}[hp]
                        for qg, n in take:
                            fillers += outproj_fillers(qg, n)
                if hp == MC - 1 and qb + 1 < NQB:
                    if qb == 0:
                        # append (not prepend): kT[3]-n3 must stay at the
                        # front to meet its scores(12) emission deadline
                        fillers += qT_group_fillers(0, qb + 1)
                    else:
                        fillers[0:0] = qT_group_fillers(0, qb + 1)
                return fillers

            # all filler lists are built up-front (closures; no emission)
            # so each call can lend its first two fillers to the previous
            # call's tail window (covering the exp(15)->ctx(15) wait).
            all_fillers = [build_fillers(i) for i in range(len(calls))]
            for idx, (qb, hp) in enumerate(calls):
                with nc.named_scope(f"c{qb}_{hp}"):
                    fillers = all_fillers[idx]
                    if idx + 1 < len(calls):
                        nqb, nhp = calls[idx + 1]
                        next_first = lambda nqb=nqb, nhp=nhp: emit_scores_exp(
                            nqb, nhp, 0
                        )
                        if F_BORROW:
                            tail_fillers = all_fillers[idx + 1][0:2]
                            del all_fillers[idx + 1][0:2]
                        else:
                            tail_fillers = ()
                    else:
                        next_first = None
                        tail_fillers = ()
                    if qb == 0 and hp == 2:
                        # wo first used from qb1-hp1; gpsimd queue
                        # keeps it off the startup DMA window.
                        nc.gpsimd.dma_start(wo_sb[:], wo_v[:])
                    if hp == 1 and qb + 1 < NQB:
                        load_qin(qb + 1)
                    # Normalize the previous call's head pair early in
                    # this call's filler stream — but not at index 0: the
                    # bcast matmul depends on the previous call's
                    # reciprocal+cast DVE chain (~0.8us), so independent
                    # fillers (two lent to the previous call's tail, two
                    # run here) keep the PE fed across the boundary.
                    if pend is not None:
                        prev = pend
                        fillers.insert(
                            min(4, len(fillers)) if F_BORROW else 0,
                            lambda prev=prev: emit_norm_finish(prev),
                        )
                    # even-spread schedule (not ceil-front-loaded): late
                    # key chunks keep filler cover for the scores sps-wait
                    nfil = len(fillers)
                    sched_l = [
                        (nfil * (k + 1)) // KC - (nfil * k) // KC
                        for k in range(KC)
                    ]
                    pend, pre = emit_attention(
                        qb, hp, fillers, per_kc=1, sched=sched_l,
                        pre=pre, next_first=next_first,
                        tail_fillers=tail_fillers,
                    )

            def emit_outproj_tail(qg, n, upto=MC, tag="ctxps", use_scalar=False, q_eng=None):
                # dc 0..upto-1 into a fresh psum group; rest + epilogue later.
                # Rides the attention's (now idle) PSUM slots; the final
                # normalize's bcast matmul keeps a free ps_proj slot.
                if tag in ("spsA", "spsB"):
                    # ride one half of an (idle) sps slot: two tail groups
                    # share each [128, 1024] scores slot
                    half = 0 if tag == "spsA" else 1
                    ps = psum.tile([P, 2 * NB], f32, name="pso", tag="sps", bufs=2)[
                        :, half * NB : (half + 1) * NB
                    ]
                else:
                    ps = psum.tile([P, NB], f32, name="pso", tag=tag, bufs=2)
                for dc in range(upto):
                    nc.tensor.matmul(
                        ps[:],
                        ctxT[dc][:, qg * P : (qg + 1) * P],
                        wo_sb[:, dc, n * NB : (n + 1) * NB],
                        start=(dc == 0),
                        stop=(dc == MC - 1),
                    )
                def finish():
                    for dc in range(upto, MC):
                        nc.tensor.matmul(
                            ps[:],
                            ctxT[dc][:, qg * P : (qg + 1) * P],
                            wo_sb[:, dc, n * NB : (n + 1) * NB],
                            start=False,
                            stop=(dc == MC - 1),
                        )
                    ot = nrm.tile([P, NB], bf16, name="ot2", tag="otile", bufs=4)
                    if use_scalar:
                        # ScalarE is idle after the final exp; the explicit
                        # dep keeps the scheduler from hoisting these casts
                        # into the exp stream (in-order queue = HOL risk).
                        ci = nc.scalar.copy(ot[:], ps[:])
                        _add_dep_helper(ci.ins, last_act["ai"].ins, sync=True, reason="tail")
                    else:
                        nc.vector.tensor_copy(ot[:], ps[:])
                    # tail stores rotate across queues: a DMA issue costs
                    # ~1.1us of queue time, so 8 serialized issues on sync
                    # would add ~9us after the last matmul
                    (q_eng or nc.sync).dma_start(
                        out_d[qg * P : (qg + 1) * P, n * NB : (n + 1) * NB], ot[:]
                    )
                return finish

            with nc.named_scope("outproj_tail"):
                # last qb's 8 outproj groups; hp 0..2's ctxT chunks are
                # normalized already, so dc 0..2 partials for 5 groups
                # (spread over the now-idle attention PSUM slots) keep the
                # PE at full clock (HAM) through the final normalize's DVE
                # chain; only the dc=3 matmul + epilogue remain per group.
                tail = [(4 * (NQB - 1) + g, n) for g in range(4) for n in range(2)]
                slots = ["ctxps", "ctxps", "spsA", "spsA", "spsB", "spsB", "ps_proj"]
                qrot = [nc.sync, nc.gpsimd, nc.scalar] if F_TAILQ else [nc.sync]
                fins = [
                    emit_outproj_tail(
                        qg, n, upto=MC - 1, tag=tag, use_scalar=(i % 2 == 0),
                        q_eng=qrot[i % len(qrot)],
                    )
                    for i, ((qg, n), tag) in enumerate(zip(tail[:7], slots))
                ]
                emit_norm_finish(pend)
                for f in fins:
                    f()
                emit_outproj_tail(*tail[7], use_scalar=True, q_eng=qrot[1 % len(qrot)])()

    nc.compile()
    return nc


def get_nc():
    if "nc" not in _BUILD_CACHE:
        _BUILD_CACHE["nc"] = _build_nc()
    return _BUILD_CACHE["nc"]


def make_in_maps(inputs):
    bf16 = ml_dtypes.bfloat16
    f32 = np.float32
    Q = np.asarray(inputs["Q"], f32)
    Q_lev = np.asarray(inputs["Q_lev"], f32)
    K = np.asarray(inputs["K"], f32)
    K_lev = np.asarray(inputs["K_lev"], f32)
    V = np.asarray(inputs["V"], f32)
    V_lev = np.asarray(inputs["V_lev"], f32)
    bq = np.asarray(inputs["bq"], f32)
    bk = np.asarray(inputs["bk"], f32)
    bv = np.asarray(inputs["bv"], f32)
    Wq = np.asarray(inputs["Wq"], f32)
    Wk = np.asarray(inputs["Wk"], f32)
    Wv = np.asarray(inputs["Wv"], f32)
    Wo = np.asarray(inputs["Wo"], f32)

    per_batch = []
    for b in range(B):
        per_batch.append(
            {
                "qt": np.ascontiguousarray(Q[b].T.astype(bf16)),
                "kt": np.ascontiguousarray(K[b].T.astype(bf16)),
                "vt": np.ascontiguousarray(V[b].T.astype(bf16)),
            }
        )
    qlevT = [np.ascontiguousarray((Q_lev[b] + bq).T).astype(bf16) for b in range(B)]
    klevT = [np.ascontiguousarray((K_lev[b] + bk).T).astype(bf16) for b in range(B)]
    vlev = [np.ascontiguousarray(V_lev[b] + bv).astype(bf16) for b in range(B)]

    in_maps = []
    for c in range(N_CORES):
        b, hh = divmod(c, 2)
        fs = slice(hh * HH, (hh + 1) * HH)
        in_maps.append(
            {
                **per_batch[b],
                "qlev": np.ascontiguousarray(qlevT[b][fs]),
                "klev": np.ascontiguousarray(klevT[b][fs]),
                "vlev": np.ascontiguousarray(vlev[b][:, fs]),
                "wq": np.ascontiguousarray(Wq[:, fs].astype(bf16)),
                "wk": np.ascontiguousarray(Wk[:, fs].astype(bf16)),
                "wv": np.ascontiguousarray(Wv[:, fs].astype(bf16)),
                "wo": np.ascontiguousarray(Wo[fs, :].astype(bf16)),
            }
        )
    return in_maps


def combine_outputs(results, inputs):
    bo = np.asarray(inputs["bo"], np.float32)
    out = np.empty((B, S, D), np.float32)
    for b in range(B):
        out[b] = (
            results[2 * b]["out"].astype(np.float32)
            + results[2 * b + 1]["out"].astype(np.float32)
            + bo
        )
    return out


def run_on_cores(inputs, trace=False):
    """Run the SPMD kernel; returns (full_output, BassKernelResults)."""
    from concourse.bass_utils import run_bass_kernel_spmd

    nc = get_nc()
    in_maps = make_in_maps(inputs)
    res = run_bass_kernel_spmd(nc, in_maps, core_ids=list(range(N_CORES)), trace=trace)
    return combine_outputs(res.results, inputs), res


def kernel(**inputs):
    out, _ = run_on_cores(inputs, trace=False)
    return out


if __name__ == "__main__":
    nc = get_nc()
    print("built + compiled OK")

